# revision 19
# baseline (speedup 1.0000x reference)
"""AutoCorrelationAttention TRN2 kernel (8-core data-parallel over batch).

Pipeline per core (2 batches):
  1. Q/K/V projections on TensorE (fp32r; Q,K with 3-pass error-feedback for
     fp32-grade accuracy -> exact top-k match vs the fp32 reference).
  2. Forward FFT of z = q + i*k per (h,d) channel via two-stage four-step
     matmul FFT (L = 2048 = 64*32), twiddles folded into per-k2 matrices.
  3. P = 2*qf*conj(kf) extracted directly from Z (Hermitian packing identity),
     head-mean accumulated by DVE tree adds.
  4. corr = inverse four-step FFT of P (matmul). mean_v = full-IDFT matmul.
  5. top-8 + indices in one DVE max8/max_index pair; softmax on ScalarE.
  6. S[h,d,f] = sum_i w_i * e^{2 pi i f delta_i / L} via one-hot matmul against
     a host phase table; all-gathered across cores (the reference broadcasts
     delay's batch axis onto the head axis, so every core needs all rows).
  7. agg = irfft(vf * S) (roll-as-phase-multiply), V FFT'd in (h-even,h-odd)
     complex pairs.
  8. out = Wo @ agg + bo on TensorE.
"""
import sys

sys.path.insert(0, "/opt/trn_rl_repo")
import numpy as np
import concourse.bacc as bacc
import concourse.bass as bass
import concourse.mybir as mybir
from concourse import tile
from concourse import bass_utils

dt = mybir.dt
F32, F32R = dt.float32, dt.float32r
ALU = mybir.AluOpType
ACTF = mybir.ActivationFunctionType

B, L, DM, H, DK = 16, 2048, 1024, 16, 64
B2 = 2            # batches per core
NCORES = 8
N1, N2 = 64, 32   # t = n1 + 64*n2 ; k = k2 + 32*k1
K1N, K2N = 64, 32
TOPK = 7
PAIRS = [(0, 16)] + [(j, 32 - j) for j in range(1, 16)]


# ----------------------------------------------------------------- host math
def rf32r(x):
    """Round fp32 -> fp32r (8-bit exp, 11-bit stored mantissa), nearest-even."""
    x = np.ascontiguousarray(x, np.float32)
    u = x.view(np.uint32)
    low = u & np.uint32(0xFFF)
    half = np.uint32(0x800)
    base = u & ~np.uint32(0xFFF)
    add = (low > half) | ((low == half) & (((u >> np.uint32(12)) & 1) == 1))
    return (base + (add.astype(np.uint32) << np.uint32(12))).view(np.float32)


def split_hl(x):
    hi = rf32r(x)
    lo = (x.astype(np.float32) - hi).astype(np.float32)
    return hi, rf32r(lo)


def build_constants():
    c = {}
    # FA [64, 64]: stage-A fwd, cols m = k2*2 + r
    FA = np.zeros((64, 64), np.float64)
    n2v = np.arange(N2)
    for k2 in range(K2N):
        th = 2 * np.pi * n2v * k2 / N2
        FA[:32, 2 * k2] = np.cos(th)
        FA[:32, 2 * k2 + 1] = -np.sin(th)
        FA[32:, 2 * k2] = np.sin(th)
        FA[32:, 2 * k2 + 1] = np.cos(th)
    c["FA_h"], c["FA_l"] = split_hl(FA)
    # FB [32][128,128]: rows [A_re n1 | A_im n1], cols m = r*64 + k1
    FB = np.zeros((K2N, 128, 128), np.float64)
    n1v = np.arange(N1)[:, None]
    k1v = np.arange(K1N)[None, :]
    for k2 in range(K2N):
        ph = 2 * np.pi * n1v * (32 * k1v + k2) / L
        wre, wim = np.cos(ph), -np.sin(ph)
        FB[k2, :64, :64] = wre
        FB[k2, :64, 64:] = wim
        FB[k2, 64:, :64] = -wim
        FB[k2, 64:, 64:] = wre
    c["FB_h"], c["FB_l"] = split_hl(FB)
    # IA [32][128,128]: inverse stage-A', rows [P_re k1 | P_im k1], cols (r,n1)
    IA = np.zeros((K2N, 128, 128), np.float64)
    s = 1.0 / (2 * L)
    k1c = np.arange(K1N)[:, None]
    n1c = np.arange(N1)[None, :]
    for k2 in range(K2N):
        ph = 2 * np.pi * n1c * (32 * k1c + k2) / L
        cc, ss = np.cos(ph), np.sin(ph)
        IA[k2, :64, :64] = s * cc
        IA[k2, :64, 64:] = s * ss
        IA[k2, 64:, :64] = -s * ss
        IA[k2, 64:, 64:] = s * cc
    c["IA"] = rf32r(IA)
    # IB [64, 32]: rows [D_re k2 | D_im k2], cols n2
    IB = np.zeros((64, 32), np.float64)
    k2c = np.arange(K2N)[:, None]
    n2c = np.arange(N2)[None, :]
    ps = 2 * np.pi * n2c * k2c / N2
    IB[:32] = np.cos(ps)
    IB[32:] = -np.sin(ps)
    c["IB"] = rf32r(IB)
    # mean IDFT [32 tiles][128, 2048]: tile s = pi*2 + r, row p: plane=p>>6,
    # k1=p&63, k2 = PAIRS[pi][plane]; value cos/-sin(2 pi k t / L)/(2*L*16)
    MID = np.zeros((32, 128, L), np.float64)
    tv = np.arange(L)[None, :]
    sm = 1.0 / (2 * L * H)
    for pi in range(16):
        for r in range(2):
            for plane in range(2):
                k2 = PAIRS[pi][plane]
                kk = k2 + 32 * np.arange(K1N)[:, None]
                ang = 2 * np.pi * kk * tv / L
                blk = np.cos(ang) if r == 0 else -np.sin(ang)
                MID[pi * 2 + r, plane * 64:(plane + 1) * 64, :] = sm * blk
    c["MID_h"], c["MID_l"] = split_hl(MID)
    # phase table T [2048 delta, 4096 f], f = k2*128 + r*64 + k1
    dv = np.arange(L)[:, None]
    kk = (np.arange(K2N)[:, None] + 32 * np.arange(K1N)[None, :]).reshape(1, -1)
    # careful: f-order is (k2, r, k1): build [2048, 32, 2, 64]
    Tt = np.zeros((L, K2N, 2, K1N), np.float64)
    ang = 2 * np.pi * dv[:, :, None] * (np.arange(K2N)[None, :, None]
          + 32 * np.arange(K1N)[None, None, :]) / L  # [2048, 32, 64]
    Tt[:, :, 0, :] = np.cos(ang)
    Tt[:, :, 1, :] = np.sin(ang)
    c["T"] = rf32r(Tt.reshape(L, 4096))
    # iota [128, 16] fp32: col cidx -> value 128*cidx + p
    c["iota"] = (np.arange(128)[:, None] + 128 * np.arange(16)[None, :]).astype(np.float32)
    c["ones1"] = rf32r(np.ones((1, 128), np.float32))
    return c


_CACHE = {}


def _build(consts_shapes):
    nc = bacc.Bacc("TRN2", target_bir_lowering=False, debug=False, num_devices=NCORES)

    # ---------------- I/O ----------------
    q_in = nc.dram_tensor("q_in", [B2, L, DM], F32, kind="ExternalInput")
    k_in = nc.dram_tensor("k_in", [B2, L, DM], F32, kind="ExternalInput")
    v_in = nc.dram_tensor("v_in", [B2, L, DM], F32, kind="ExternalInput")
    WqT_h = nc.dram_tensor("WqT_h", [DM, DM], F32R, kind="ExternalInput")
    WqT_l = nc.dram_tensor("WqT_l", [DM, DM], F32R, kind="ExternalInput")
    WkT_h = nc.dram_tensor("WkT_h", [DM, DM], F32R, kind="ExternalInput")
    WkT_l = nc.dram_tensor("WkT_l", [DM, DM], F32R, kind="ExternalInput")
    WvT = nc.dram_tensor("WvT", [DM, DM], F32R, kind="ExternalInput")
    WoT = nc.dram_tensor("WoT", [DM, DM], F32R, kind="ExternalInput")
    bo_in = nc.dram_tensor("bo_in", [DM], F32, kind="ExternalInput")
    FAh = nc.dram_tensor("FA_h", [64, 64], F32R, kind="ExternalInput")
    FAl = nc.dram_tensor("FA_l", [64, 64], F32R, kind="ExternalInput")
    FBh = nc.dram_tensor("FB_h", [K2N, 128, 128], F32R, kind="ExternalInput")
    FBl = nc.dram_tensor("FB_l", [K2N, 128, 128], F32R, kind="ExternalInput")
    IAt = nc.dram_tensor("IA", [K2N, 128, 128], F32R, kind="ExternalInput")
    IBt = nc.dram_tensor("IB", [64, 32], F32R, kind="ExternalInput")
    MIDh = nc.dram_tensor("MID_h", [32, 128, L], F32R, kind="ExternalInput")
    MIDl = nc.dram_tensor("MID_l", [32, 128, L], F32R, kind="ExternalInput")
    Tt = nc.dram_tensor("T", [L, 4096], F32R, kind="ExternalInput")
    iota_in = nc.dram_tensor("iota", [128, 16], F32, kind="ExternalInput")
    ones1 = nc.dram_tensor("ones1", [1, 128], F32R, kind="ExternalInput")

    corr_out = nc.dram_tensor("corr", [B2, DM, L], F32, kind="ExternalOutput")
    out_out = nc.dram_tensor("out", [B2, L, DM], F32, kind="ExternalOutput")

    # ---------------- scratch ----------------
    qt = nc.dram_tensor("qt", [DM, B2, L], F32, kind="Internal")
    kt = nc.dram_tensor("kt", [DM, B2, L], F32, kind="Internal")
    vt = nc.dram_tensor("vt", [DM, B2, L], F32R, kind="Internal")
    bufA = nc.dram_tensor("bufA", [B2, K2N, 2, DM, N1], F32, kind="Internal")
    bufZ = nc.dram_tensor("bufZ", [B2, K2N, 128, DM], F32, kind="Internal")
    bufAv = nc.dram_tensor("bufAv", [B2, K2N, 2, 512, N1], F32R, kind="Internal")
    bufZv = nc.dram_tensor("bufZv", [B2, K2N, 128, 512], F32R, kind="Internal")
    bufP = nc.dram_tensor("bufP", [B2, 2, K2N, K1N, DM], F32, kind="Internal")
    bufD = nc.dram_tensor("bufD", [B2, K2N, 2, DM, N1], F32R, kind="Internal")
    bufAF = nc.dram_tensor("bufAF", [B2, 2, K2N, K1N, DM], F32R, kind="Internal")
    bufDa = nc.dram_tensor("bufDa", [B2, K2N, 2, DM, N1], F32R, kind="Internal")
    bufAgg = nc.dram_tensor("bufAgg", [B2, DM, L], F32R, kind="Internal")
    repbuf = nc.dram_tensor("repbuf", [2, TOPK, 128], F32R, kind="Internal")

    def dap(t, off, dims):
        a = t.ap()
        return bass.AP(tensor=a.tensor, offset=off, ap=[list(d) for d in dims])

    ES = 1  # strides in elements

    with tile.TileContext(nc, num_cores=NCORES) as tc:
        # ======== phase 1: projections ========
        def projection(x_in, dst, w_h, w_l, x3):
            with tc.tile_pool(name="pw", bufs=1) as pw, \
                 tc.tile_pool(name="pxl", bufs=9) as pxl, \
                 tc.tile_pool(name="px", bufs=3) as px, \
                 tc.tile_pool(name="pst", bufs=4) as pst, \
                 tc.tile_pool(name="pps", bufs=4, space="PSUM") as pps:
                wh = pw.tile([128, 8 * DM], F32R, tag="wh")
                for kc in range(8):
                    nc.sync.dma_start(wh[:, kc * DM:(kc + 1) * DM],
                                      w_h.ap()[kc * 128:(kc + 1) * 128, :])
                if x3:
                    wl = pw.tile([128, 8 * DM], F32R, tag="wl")
                    for kc in range(8):
                        nc.sync.dma_start(wl[:, kc * DM:(kc + 1) * DM],
                                          w_l.ap()[kc * 128:(kc + 1) * 128, :])
                for b in range(B2):
                    for nchunk in range(4):
                        t0 = nchunk * 512
                        rhs_h, rhs_l = [], []
                        for kc in range(8):
                            if x3:
                                xt = px.tile([128, 512], F32, tag="xt")
                                src = dap(x_in, b * L * DM + t0 * DM + kc * 128,
                                          [[ES, 128], [DM, 512]])
                                nc.sync.dma_start(xt[:], src)
                                ht = pxl.tile([128, 512], F32R, tag="ht")
                                lt = pxl.tile([128, 512], F32R, tag="lt")
                                nc.vector.tensor_copy(ht[:], xt[:])
                                nc.vector.tensor_sub(lt[:], xt[:], ht[:])
                                rhs_h.append(ht)
                                rhs_l.append(lt)
                            else:
                                ht = pxl.tile([128, 512], F32R, tag="ht")
                                src = dap(x_in, b * L * DM + t0 * DM + kc * 128,
                                          [[ES, 128], [DM, 512]])
                                nc.sync.dma_start(ht[:], src.bitcast(F32R))
                                rhs_h.append(ht)
                        for mc in range(8):
                            ps = pps.tile([128, 512], F32, tag="ps")
                            for kc in range(8):
                                lh = wh[:, kc * DM + mc * 128:kc * DM + (mc + 1) * 128]
                                st = (kc == 0)
                                if x3:
                                    ll = wl[:, kc * DM + mc * 128:kc * DM + (mc + 1) * 128]
                                    nc.tensor.matmul(ps[:], lh, rhs_h[kc][:], start=st, stop=False)
                                    nc.tensor.matmul(ps[:], lh, rhs_l[kc][:], start=False, stop=False)
                                    nc.tensor.matmul(ps[:], ll, rhs_h[kc][:], start=False,
                                                     stop=(kc == 7))
                                else:
                                    nc.tensor.matmul(ps[:], lh, rhs_h[kc][:], start=st,
                                                     stop=(kc == 7))
                            ot = pst.tile([128, 512], dst.dtype, tag="ot")
                            nc.scalar.activation(ot[:], ps[:], ACTF.Copy)
                            d = dap(dst, (mc * 128) * (B2 * L) + b * L + t0,
                                    [[B2 * L, 128], [ES, 512]])
                            nc.sync.dma_start(d, ot[:])

        projection(q_in, qt, WqT_h, WqT_l, True)
        projection(k_in, kt, WkT_h, WkT_l, True)
        projection(v_in, vt, WvT, WvT, False)

        # ======== phase 2: forward FFT of z = q + i k (x3) ========
        with tc.tile_pool(name="fw", bufs=1) as fw, \
             tc.tile_pool(name="fx", bufs=3) as fx, \
             tc.tile_pool(name="fst", bufs=4) as fst, \
             tc.tile_pool(name="fps", bufs=4, space="PSUM") as fps:
            fah = fw.tile([64, 64], F32R, tag="fah")
            fal = fw.tile([64, 64], F32R, tag="fal")
            nc.sync.dma_start(fah[:], FAh.ap()[:])
            nc.sync.dma_start(fal[:], FAl.ap()[:])
            # stage A: per (b, col-chunk of (ch8, n1-64))
            for b in range(B2):
                for cc in range(128):
                    ch0 = cc * 8
                    xt = fx.tile([64, 512], F32, tag="axt")
                    for half, srcT in ((0, qt), (1, kt)):
                        src = dap(srcT, ch0 * (B2 * L) + b * L,
                                  [[N1, 32], [B2 * L, 8], [ES, 64]])
                        nc.sync.dma_start(xt[half * 32:(half + 1) * 32, :], src)
                    ht = fx.tile([64, 512], F32R, tag="aht")
                    lt = fx.tile([64, 512], F32R, tag="alt")
                    nc.vector.tensor_copy(ht[:], xt[:])
                    nc.vector.tensor_sub(lt[:], xt[:], ht[:])
                    ps = fps.tile([64, 512], F32, tag="ps")
                    nc.tensor.matmul(ps[:], fah[:], ht[:], start=True, stop=False)
                    nc.tensor.matmul(ps[:], fah[:], lt[:], start=False, stop=False)
                    nc.tensor.matmul(ps[:], fal[:], ht[:], start=False, stop=True)
                    ot = fst.tile([64, 512], F32, tag="aot")
                    nc.scalar.activation(ot[:], ps[:], ACTF.Copy)
                    d = dap(bufA, ((b * K2N * 2) * DM + ch0) * N1,
                            [[DM * N1, 64], [N1, 8], [ES, 64]])
                    nc.sync.dma_start(d, ot[:])
            # stage B: per (b, k2): rhs [128 (r,n1), 1024 ch]
            for k2 in range(K2N):
                fbh = fw.tile([128, 128], F32R, tag="fbh")
                fbl = fw.tile([128, 128], F32R, tag="fbl")
                nc.sync.dma_start(fbh[:], FBh.ap()[k2, :, :])
                nc.sync.dma_start(fbl[:], FBl.ap()[k2, :, :])
                for b in range(B2):
                    xt = fx.tile([128, 1024], F32, tag="bxt")
                    for r in range(2):
                        src = dap(bufA, (((b * K2N + k2) * 2 + r) * DM) * N1,
                                  [[ES, 64], [N1, 1024]])
                        nc.sync.dma_start(xt[r * 64:(r + 1) * 64, :], src)
                    ht = fx.tile([128, 1024], F32R, tag="bht")
                    lt = fx.tile([128, 1024], F32R, tag="blt")
                    nc.vector.tensor_copy(ht[:], xt[:])
                    nc.vector.tensor_sub(lt[:], xt[:], ht[:])
                    for half in range(2):
                        sl = slice(half * 512, (half + 1) * 512)
                        ps = fps.tile([128, 512], F32, tag="ps")
                        nc.tensor.matmul(ps[:], fbh[:], ht[:, sl], start=True, stop=False)
                        nc.tensor.matmul(ps[:], fbh[:], lt[:, sl], start=False, stop=False)
                        nc.tensor.matmul(ps[:], fbl[:], ht[:, sl], start=False, stop=True)
                        ot = fst.tile([128, 512], F32, tag="bot")
                        nc.scalar.activation(ot[:], ps[:], ACTF.Copy)
                        d = dap(bufZ, ((b * K2N + k2) * 128) * DM + half * 512,
                                [[DM, 128], [ES, 512]])
                        nc.sync.dma_start(d, ot[:])

        # ======== phase 3: product P (ch-on-partitions, f-on-free) ========
        mtiles = []
        with tc.tile_pool(name="pm", bufs=1) as pm:
            for s in range(32):
                mt = pm.tile([128, 128], F32, tag=f"mt{s}")
                mtiles.append(mt)
            with tc.tile_pool(name="pu", bufs=2) as pu, \
                 tc.tile_pool(name="pt", bufs=2) as pt:
                PL = K2N * 128 * DM  # k2-plane stride in bufZ elements (per b)

                def fslice(t, r, kstride=128):
                    a = t[:]
                    return bass.AP(tensor=a.tensor, offset=a.offset + r * 64,
                                   ap=[list(a.ap[0]), [kstride, 32], [1, 64]])

                def load_uw(u, w, srcbuf, b, ch0, chw, K2S):
                    # u: straight [128, 4096]; w: mirrored+reversed
                    a = srcbuf.ap()
                    base = b * K2N * 128 * chw + ch0
                    nc.sync.dma_start(
                        u[:], bass.AP(tensor=a.tensor, offset=base,
                                      ap=[[1, 128], [128 * chw, 32], [chw, 128]]))
                    # w main: k2 1..31 reversed, per r, k1 reversed
                    for r in range(2):
                        for k2d in range(1, 32):
                            dst = bass.AP(tensor=w[:].tensor,
                                          offset=w[:].offset + k2d * 128 + r * 64,
                                          ap=[list(w[:].ap[0]), [1, 64]])
                            s0 = base + (32 - k2d) * 128 * chw + (r * 64 + 63) * chw
                            nc.sync.dma_start(
                                dst, bass.AP(tensor=a.tensor, offset=s0,
                                             ap=[[1, 128], [-chw, 64]]))
                    # w k2=0: k1'=(64-k1)%64
                    for r in range(2):
                        s00 = base + (r * 64) * chw
                        dst0 = bass.AP(tensor=w[:].tensor,
                                       offset=w[:].offset + r * 64,
                                       ap=[list(w[:].ap[0]), [1, 1]])
                        nc.sync.dma_start(
                            dst0, bass.AP(tensor=a.tensor, offset=s00,
                                          ap=[[1, 128], [chw, 1]]))
                        dst1 = bass.AP(tensor=w[:].tensor,
                                       offset=w[:].offset + r * 64 + 1,
                                       ap=[list(w[:].ap[0]), [1, 63]])
                        nc.sync.dma_start(
                            dst1, bass.AP(tensor=a.tensor, offset=s00 + 63 * chw,
                                          ap=[[1, 128], [-chw, 63]]))

                for b in range(B2):
                    for cc in range(8):
                        ch0 = cc * 128
                        u = pu.tile([128, 4096], F32, tag="u")
                        w = pu.tile([128, 4096], F32, tag="w")
                        load_uw(u, w, bufZ, b, ch0, DM, K2N)
                        pre = pt.tile([128, 2048], F32, tag="pre")
                        pim = pt.tile([128, 2048], F32, tag="pim")
                        tq = pt.tile([128, 2048], F32, tag="tq")
                        ure, uim = fslice(u, 0), fslice(u, 1)
                        wre, wim = fslice(w, 0), fslice(w, 1)
                        prs = fslice(pre, 0, 64)
                        pis = fslice(pim, 0, 64)
                        tqs = fslice(tq, 0, 64)
                        # P_re = ur*wi + ui*wr
                        nc.vector.tensor_mul(prs, ure, wim)
                        nc.vector.tensor_mul(tqs, uim, wre)
                        nc.vector.tensor_add(prs, prs, tqs)
                        # P_im = (ur^2+ui^2-wr^2-wi^2)/2
                        nc.vector.tensor_mul(pis, ure, ure)
                        nc.vector.tensor_mul(tqs, uim, uim)
                        nc.vector.tensor_add(pis, pis, tqs)
                        nc.vector.tensor_mul(tqs, wre, wre)
                        nc.vector.tensor_sub(pis, pis, tqs)
                        nc.vector.tensor_mul(tqs, wim, wim)
                        nc.vector.tensor_sub(pis, pis, tqs)
                        nc.vector.tensor_scalar_mul(pim[:], pim[:], 0.5)
                        # write P [128 ch, (k2,k1)] per r to bufP fp32
                        pa = bufP.ap()
                        for r, tl in ((0, pre), (1, pim)):
                            d = bass.AP(tensor=pa.tensor,
                                        offset=(b * 2 + r) * K2N * K1N * DM + ch0,
                                        ap=[[1, 128], [DM, 2048]])
                            nc.sync.dma_start(d, tl[:])
                # mean tiles: reload P with (plane,k1) on partitions
                for b in range(B2):
                    for pi in range(16):
                        k2a, k2b = PAIRS[pi]
                        for r in range(2):
                            xt = pu.tile([128, 1024], F32, tag="mx")
                            pa = bufP.ap()
                            for pl, k2 in ((0, k2a), (1, k2b)):
                                s0 = (b * 2 + r) * K2N * K1N * DM + k2 * K1N * DM
                                nc.sync.dma_start(
                                    xt[pl * 64:(pl + 1) * 64, :],
                                    bass.AP(tensor=pa.tensor, offset=s0,
                                            ap=[[DM, 64], [1, 1024]]))
                            t1 = pt.tile([128, 512], F32, tag="t1")
                            nc.vector.tensor_add(t1[:], xt[:, :512], xt[:, 512:])
                            t2 = pt.tile([128, 256], F32, tag="t2")
                            nc.vector.tensor_add(t2[:], t1[:, :256], t1[:, 256:])
                            t3 = pt.tile([128, 128], F32, tag="t3")
                            nc.vector.tensor_add(t3[:], t2[:, :128], t2[:, 128:])
                            nc.vector.tensor_add(
                                mtiles[pi * 2 + r][:, b * 64:(b + 1) * 64],
                                t3[:, :64], t3[:, 64:])

            # ======== phase 4: corr inverse ========
            with tc.tile_pool(name="iw", bufs=1) as iw, \
                 tc.tile_pool(name="ix", bufs=2) as ix, \
                 tc.tile_pool(name="ist", bufs=2) as ist, \
                 tc.tile_pool(name="sxp", bufs=1) as sxp, \
                 tc.tile_pool(name="p8", bufs=1) as p8, \
                 tc.tile_pool(name="dpool", bufs=1, space="DRAM") as dpool, \
                 tc.tile_pool(name="ips", bufs=4, space="PSUM") as ips:
                S_loc_t = dpool.tile([128, 4096], F32R, tag="sloc")
                S_full_t = dpool.tile([128 * NCORES, 4096], F32R, tag="sfull")
                ibt = iw.tile([64, 32], F32R, tag="ibt")
                nc.sync.dma_start(ibt[:], IBt.ap()[:])

                def inverse(srcP, srcD, dstD, writer):
                    # stage A'
                    for k2 in range(K2N):
                        iat = iw.tile([128, 128], F32R, tag="iat")
                        nc.sync.dma_start(iat[:], IAt.ap()[k2, :, :])
                        for b in range(B2):
                            cw = srcP.shape[-1]
                            xt = ix.tile([128, cw], F32R, tag="ixt")
                            for r in range(2):
                                src = dap(srcP,
                                          (b * 2 + r) * K2N * K1N * cw
                                          + k2 * K1N * cw,
                                          [[cw, 64], [ES, cw]])
                                nc.sync.dma_start(xt[r * 64:(r + 1) * 64, :],
                                                  src.bitcast(F32R))
                            for half in range(cw // 512):
                                sl = slice(half * 512, (half + 1) * 512)
                                ps = ips.tile([128, 512], F32, tag="ps")
                                nc.tensor.matmul(ps[:], iat[:], xt[:, sl],
                                                 start=True, stop=True)
                                ot = ist.tile([128, 512], F32R, tag="iot")
                                nc.scalar.activation(ot[:], ps[:], ACTF.Copy)
                                for r in range(2):
                                    d = dap(srcD,
                                            (((b * K2N + k2) * 2 + r) * cw) * N1
                                            + half * 512 * N1,
                                            [[ES, 64], [N1, 512]])
                                    nc.sync.dma_start(d, ot[r * 64:(r + 1) * 64, :])
                    # stage B'
                    for b in range(B2):
                        cw = srcP.shape[-1]
                        nch = cw // 64  # ch per 4096-col tile piece
                        for piece in range(cw * N1 // 1024):
                            xt = ix.tile([64, 1024], F32R, tag="ibx")
                            ch0 = piece * 16
                            for r in range(2):
                                src = dap(srcD,
                                          ((b * K2N) * 2 + r) * cw * N1 + ch0 * N1,
                                          [[2 * cw * N1, 32], [N1, 16], [ES, 64]])
                                nc.sync.dma_start(xt[r * 32:(r + 1) * 32, :], src)
                            for half in range(2):
                                sl = slice(half * 512, (half + 1) * 512)
                                ps = ips.tile([32, 512], F32, tag="ps")
                                nc.tensor.matmul(ps[:], ibt[:], xt[:, sl],
                                                 start=True, stop=True)
                                ot = ist.tile([32, 512], writer_dtype, tag="iot2")
                                nc.scalar.activation(ot[:], ps[:], ACTF.Copy)
                                writer(b, ch0 + half * 8, ot)

                writer_dtype = F32

                def wr_corr(b, ch0, ot):
                    d = dap(corr_out, (b * DM + ch0) * L,
                            [[N1, 32], [L, 8], [ES, 64]])
                    nc.sync.dma_start(d, ot[:])

                inverse(bufP, bufD, None, wr_corr)

                # ======== mean path: full IDFT matmul ========
                with tc.tile_pool(name="mm", bufs=1) as mm, \
                     tc.tile_pool(name="mq", bufs=1) as mq:
                    meanv = mm.tile([128, 2048], F32, tag="meanv")
                    for tchunk in range(4):
                        ps = ips.tile([128, 512], F32, tag="ps")
                        for s in range(32):
                            rh = mq.tile([128, 512], F32R, tag="rh")
                            rl = mq.tile([128, 512], F32R, tag="rl")
                            nc.sync.dma_start(rh[:], MIDh.ap()[s, :, tchunk * 512:(tchunk + 1) * 512])
                            nc.sync.dma_start(rl[:], MIDl.ap()[s, :, tchunk * 512:(tchunk + 1) * 512])
                            hh = mq.tile([128, 128], F32R, tag="mhh")
                            ll = mq.tile([128, 128], F32R, tag="mhl")
                            nc.vector.tensor_copy(hh[:], mtiles[s][:])
                            nc.vector.tensor_sub(ll[:], mtiles[s][:], hh[:])
                            nc.tensor.matmul(ps[:], hh[:], rh[:], start=(s == 0), stop=False)
                            nc.tensor.matmul(ps[:], hh[:], rl[:], start=False, stop=False)
                            nc.tensor.matmul(ps[:], ll[:], rh[:], start=False, stop=(s == 31))
                        nc.scalar.activation(meanv[:, tchunk * 512:(tchunk + 1) * 512],
                                             ps[:], ACTF.Copy)

                    # ======== top-k, softmax, one-hot, S ========
                    mx = mm.tile([128, 8], F32, tag="mx")
                    mi = mm.tile([128, 8], dt.uint32, tag="mi")
                    nc.vector.max(mx[:], meanv[:])
                    nc.vector.max_index(mi[:], mx[:], meanv[:])
                    delt = mm.tile([128, 8], F32, tag="delt")
                    nc.vector.tensor_copy(delt[:], mi[:])
                    negm = mm.tile([128, 1], F32, tag="negm")
                    nc.scalar.mul(negm[:], mx[:, 0:1], -1.0)
                    ex = mm.tile([128, 7], F32, tag="ex")
                    nc.scalar.activation(ex[:], mx[:, 0:7], ACTF.Exp, bias=negm[:], scale=1.0)
                    sm = mm.tile([128, 1], F32, tag="sm")
                    nc.vector.tensor_reduce(sm[:], ex[:], axis=mybir.AxisListType.X, op=ALU.add)
                    rs = mm.tile([128, 1], F32, tag="rs")
                    nc.vector.reciprocal(rs[:], sm[:])
                    wt = mm.tile([128, 7], F32, tag="wt")
                    nc.vector.tensor_scalar_mul(wt[:], ex[:], rs[:])
                    # stage delta/w rows to DRAM then replicate via ones-matmul
                    for i in range(TOPK):
                        nc.sync.dma_start(
                            dap(repbuf, (0 * TOPK + i) * 128, [[ES, 128]]),
                            delt[:, i:i + 1].bitcast(F32R))
                        nc.sync.dma_start(
                            dap(repbuf, (1 * TOPK + i) * 128, [[ES, 128]]),
                            wt[:, i:i + 1].bitcast(F32R))
                    onr = mm.tile([1, 128], F32R, tag="onr")
                    nc.sync.dma_start(onr[:], ones1.ap()[:])
                    iot = mm.tile([128, 16], F32, tag="iot3")
                    nc.sync.dma_start(iot[:], iota_in.ap()[:])
                    reps = []
                    for i in range(TOPK):
                        for j in range(2):
                            rowt = mq.tile([1, 128], F32R, tag="rowt")
                            nc.sync.dma_start(rowt[:], dap(repbuf, (j * TOPK + i) * 128,
                                                           [[128, 1], [ES, 128]]))
                            ps = ips.tile([128, 128], F32, tag="ps")
                            nc.tensor.matmul(ps[:], onr[:], rowt[:], start=True, stop=True)
                            rt = mm.tile([128, 128], F32, tag=f"rep{i}_{j}")
                            nc.scalar.activation(rt[:], ps[:], ACTF.Copy)
                            reps.append(rt)
                    # one-hot chunks + S matmul
                    ohs = []
                    for cc in range(16):
                        oh = mm.tile([128, 128], F32R, tag=f"oh{cc}")
                        tmp = mq.tile([128, 128], F32R, tag="ohtmp")
                        for i in range(TOPK):
                            drep, wrep = reps[i * 2], reps[i * 2 + 1]
                            dst = oh if i == 0 else tmp
                            nc.vector.scalar_tensor_tensor(
                                dst[:], drep[:], iot[:, cc:cc + 1], wrep[:],
                                op0=ALU.is_equal, op1=ALU.mult)
                            if i > 0:
                                nc.vector.tensor_add(oh[:], oh[:], tmp[:])
                        ohs.append(oh)
                    for fc in range(8):
                        ps = ips.tile([128, 512], F32, tag="ps")
                        for cc in range(16):
                            rt = mq.tile([128, 512], F32R, tag="trt")
                            nc.sync.dma_start(
                                rt[:], Tt.ap()[cc * 128:(cc + 1) * 128,
                                               fc * 512:(fc + 1) * 512])
                            nc.tensor.matmul(ps[:], ohs[cc][:], rt[:],
                                             start=(cc == 0), stop=(cc == 15))
                        st = mq.tile([128, 512], F32R, tag="sst")
                        nc.scalar.activation(st[:], ps[:], ACTF.Copy)
                        nc.sync.dma_start(S_loc_t[:, fc * 512:(fc + 1) * 512], st[:])

                # all-gather S across the 8 cores
                nc.gpsimd.collective_compute(
                    "AllGather", ALU.bypass,
                    replica_groups=[list(range(NCORES))],
                    ins=[S_loc_t.opt()],
                    outs=[S_full_t.opt()],
                )

                # ======== phase 7: V forward FFT (x1) ========
                fah2 = iw.tile([64, 64], F32R, tag="fah2")
                nc.sync.dma_start(fah2[:], FAh.ap()[:])
                for b in range(B2):
                    for cc in range(64):
                        c0 = cc * 8  # c2 = (hp, d): 8 per chunk
                        xt = ix.tile([64, 512], F32R, tag="vxt")
                        for half in range(2):  # half 0: even h, 1: odd h
                            # c2 = hp*64 + d ; ch = (2*hp+half)*64 + d
                            src = dap(vt,
                                      ((((c0 // 64) * 2 + half) * 64 + (c0 % 64))
                                       * (B2 * L)) + b * L,
                                      [[N1, 32], [B2 * L, 8], [ES, 64]])
                            nc.sync.dma_start(xt[half * 32:(half + 1) * 32, :], src)
                        ps = ips.tile([64, 512], F32, tag="ps")
                        nc.tensor.matmul(ps[:], fah2[:], xt[:], start=True, stop=True)
                        ot = ist.tile([64, 512], F32R, tag="vot")
                        nc.scalar.activation(ot[:], ps[:], ACTF.Copy)
                        d = dap(bufAv, ((b * K2N * 2) * 512 + c0) * N1,
                                [[512 * N1, 64], [N1, 8], [ES, 64]])
                        nc.sync.dma_start(d, ot[:])
                    for k2 in range(K2N):
                        fbh2 = iw.tile([128, 128], F32R, tag="fbh2")
                        nc.sync.dma_start(fbh2[:], FBh.ap()[k2, :, :])
                        xt = ix.tile([128, 512], F32R, tag="vbx")
                        for r in range(2):
                            src = dap(bufAv, (((b * K2N + k2) * 2 + r) * 512) * N1,
                                      [[ES, 64], [N1, 512]])
                            nc.sync.dma_start(xt[r * 64:(r + 1) * 64, :], src)
                        ps = ips.tile([128, 512], F32, tag="ps")
                        nc.tensor.matmul(ps[:], fbh2[:], xt[:], start=True, stop=True)
                        ot = ist.tile([128, 512], F32R, tag="vot2")
                        nc.scalar.activation(ot[:], ps[:], ACTF.Copy)
                        d = dap(bufZv, ((b * K2N + k2) * 128) * 512,
                                [[512, 128], [ES, 512]])
                        nc.sync.dma_start(d, ot[:])

                # ======== phase 8: AF = 2*vf*S, (ch-part, f-free) ========
                # S_full row order: core c wrote rows (b_loc, d) at c*128 +
                # b_loc*64 + d; global h = c*2 + b_loc, so row = h*64 + d.
                PLv = 128 * 512  # k2-plane stride of bufZv per b

                def fsl(t, r, kstride=128, off0=0):
                    a = t[:]
                    return bass.AP(tensor=a.tensor, offset=a.offset + off0 + r * 64,
                                   ap=[list(a.ap[0]), [kstride, 32], [1, 64]])

                def load_uw_v(u, w, b, c0):
                    a = bufZv.ap()
                    base = b * K2N * 128 * 512 + c0
                    nc.sync.dma_start(
                        u[:], bass.AP(tensor=a.tensor, offset=base,
                                      ap=[[1, 128], [128 * 512, 32], [512, 128]]))
                    for r in range(2):
                        for k2d in range(1, 32):
                            dst = bass.AP(tensor=w[:].tensor,
                                          offset=w[:].offset + k2d * 128 + r * 64,
                                          ap=[list(w[:].ap[0]), [1, 64]])
                            s0 = base + (32 - k2d) * 128 * 512 + (r * 64 + 63) * 512
                            nc.sync.dma_start(
                                dst, bass.AP(tensor=a.tensor, offset=s0,
                                             ap=[[1, 128], [-512, 64]]))
                    for r in range(2):
                        s00 = base + (r * 64) * 512
                        dst0 = bass.AP(tensor=w[:].tensor,
                                       offset=w[:].offset + r * 64,
                                       ap=[list(w[:].ap[0]), [1, 1]])
                        nc.sync.dma_start(
                            dst0, bass.AP(tensor=a.tensor, offset=s00,
                                          ap=[[1, 128], [512, 1]]))
                        dst1 = bass.AP(tensor=w[:].tensor,
                                       offset=w[:].offset + r * 64 + 1,
                                       ap=[list(w[:].ap[0]), [1, 63]])
                        nc.sync.dma_start(
                            dst1, bass.AP(tensor=a.tensor, offset=s00 + 63 * 512,
                                          ap=[[1, 128], [-512, 63]]))

                sfap = S_full_t[:]
                for cc in range(4):
                    c0 = cc * 128
                    se = sxp.tile([128, 4096], F32R, tag="se")
                    so = sxp.tile([128, 4096], F32R, tag="so")
                    for hpl in range(2):
                        hp = 2 * cc + hpl
                        nc.sync.dma_start(
                            se[hpl * 64:(hpl + 1) * 64, :],
                            bass.AP(tensor=sfap.tensor,
                                    offset=sfap.offset + (2 * hp) * 64 * 4096,
                                    ap=[[4096, 64], [1, 4096]]))
                        nc.sync.dma_start(
                            so[hpl * 64:(hpl + 1) * 64, :],
                            bass.AP(tensor=sfap.tensor,
                                    offset=sfap.offset + (2 * hp + 1) * 64 * 4096,
                                    ap=[[4096, 64], [1, 4096]]))
                    for b in range(B2):
                        u = p8.tile([128, 4096], F32R, tag="vu")
                        w = p8.tile([128, 4096], F32R, tag="vw")
                        load_uw_v(u, w, b, c0)
                        for par in range(2):
                            st = se if par == 0 else so
                            sre, sie = fsl(st, 0), fsl(st, 1)
                            evr = p8.tile([128, 2048], F32, tag="evr")
                            evi = p8.tile([128, 2048], F32, tag="evi")
                            zr, zi = fsl(u, 0), fsl(u, 1)
                            wzr, wzi = fsl(w, 0), fsl(w, 1)
                            vrs, vis = fsl(evr, 0, 64), fsl(evi, 0, 64)
                            if par == 0:
                                nc.vector.tensor_add(vrs, zr, wzr)
                                nc.vector.tensor_sub(vis, zi, wzi)
                            else:
                                nc.vector.tensor_add(vrs, zi, wzi)
                                nc.vector.tensor_sub(vis, wzr, zr)
                            are = p8.tile([128, 2048], F32R, tag="are")
                            aim = p8.tile([128, 2048], F32R, tag="aim")
                            tq2 = p8.tile([128, 2048], F32, tag="tq2")
                            tq3 = p8.tile([128, 2048], F32, tag="tq3")
                            t2s, t3s = fsl(tq2, 0, 64), fsl(tq3, 0, 64)
                            nc.vector.tensor_mul(t2s, vrs, sre)
                            nc.vector.tensor_mul(t3s, vis, sie)
                            nc.vector.tensor_sub(fsl(are, 0, 64), t2s, t3s)
                            nc.vector.tensor_mul(t2s, vrs, sie)
                            nc.vector.tensor_mul(t3s, vis, sre)
                            nc.vector.tensor_add(fsl(aim, 0, 64), t2s, t3s)
                            afap = bufAF.ap()
                            for r, tl in ((0, are), (1, aim)):
                                for hpl in range(2):
                                    hp = 2 * cc + hpl
                                    ch0 = (2 * hp + par) * 64
                                    d = bass.AP(
                                        tensor=afap.tensor,
                                        offset=(b * 2 + r) * K2N * K1N * DM + ch0,
                                        ap=[[1, 64], [DM, 2048]])
                                    nc.sync.dma_start(
                                        d, tl[hpl * 64:(hpl + 1) * 64, :])

                def wr_agg(b, ch0, ot):
                    d = dap(bufAgg, (b * DM + ch0) * L,
                            [[N1, 32], [L, 8], [ES, 64]])
                    nc.sync.dma_start(d, ot[:])

                writer_dtype = F32R
                inverse(bufAF, bufDa, None, wr_agg)

        # ======== phase 9: output projection ========
        with tc.tile_pool(name="ow", bufs=1) as ow, \
             tc.tile_pool(name="ox", bufs=9) as ox, \
             tc.tile_pool(name="ost", bufs=4) as ost, \
             tc.tile_pool(name="ops", bufs=4, space="PSUM") as ops:
            woT = ow.tile([128, 8 * DM], F32R, tag="woT")
            for kc in range(8):
                nc.sync.dma_start(woT[:, kc * DM:(kc + 1) * DM],
                                  WoT.ap()[kc * 128:(kc + 1) * 128, :])
            bot = ow.tile([128, 8], F32, tag="bot")
            nc.sync.dma_start(bot[:], dap(bo_in, 0, [[ES, 128], [128, 8]]))
            for b in range(B2):
                for tchunk in range(4):
                    t0 = tchunk * 512
                    rhs = []
                    for kc in range(8):
                        rt = ox.tile([128, 512], F32R, tag="ort")
                        src = dap(bufAgg, (b * DM + kc * 128) * L + t0,
                                  [[L, 128], [ES, 512]])
                        nc.sync.dma_start(rt[:], src)
                        rhs.append(rt)
                    for mc in range(8):
                        ps = ops.tile([128, 512], F32, tag="ps")
                        for kc in range(8):
                            lh = woT[:, kc * DM + mc * 128:kc * DM + (mc + 1) * 128]
                            nc.tensor.matmul(ps[:], lh, rhs[kc][:],
                                             start=(kc == 0), stop=(kc == 7))
                        ot = ost.tile([128, 512], F32, tag="oot")
                        nc.scalar.activation(ot[:], ps[:], ACTF.Identity,
                                             bias=bot[:, mc:mc + 1], scale=1.0)
                        d = dap(out_out, b * L * DM + t0 * DM + mc * 128,
                                [[ES, 128], [DM, 512]])
                        nc.sync.dma_start(d, ot[:])

    nc.compile()
    return nc


def kernel(queries, keys, values, Wq, Wk, Wv, Wo, bo):
    queries = np.asarray(queries, np.float32)
    keys = np.asarray(keys, np.float32)
    values = np.asarray(values, np.float32)
    if "nc" not in _CACHE:
        _CACHE["consts"] = build_constants()
        _CACHE["nc"] = _build(None)
    nc = _CACHE["nc"]
    c = _CACHE["consts"]
    WqTh, WqTl = split_hl(np.asarray(Wq, np.float32).T)
    WkTh, WkTl = split_hl(np.asarray(Wk, np.float32).T)
    WvT = rf32r(np.ascontiguousarray(np.asarray(Wv, np.float32).T))
    WoT = rf32r(np.ascontiguousarray(np.asarray(Wo, np.float32).T))
    base = {
        "WqT_h": np.ascontiguousarray(WqTh), "WqT_l": np.ascontiguousarray(WqTl),
        "WkT_h": np.ascontiguousarray(WkTh), "WkT_l": np.ascontiguousarray(WkTl),
        "WvT": WvT, "WoT": WoT, "bo_in": np.asarray(bo, np.float32),
        "FA_h": c["FA_h"], "FA_l": c["FA_l"], "FB_h": c["FB_h"], "FB_l": c["FB_l"],
        "IA": c["IA"], "IB": c["IB"], "MID_h": c["MID_h"], "MID_l": c["MID_l"],
        "T": c["T"], "iota": c["iota"], "ones1": c["ones1"],
    }
    in_maps = []
    for core in range(NCORES):
        sl = slice(core * B2, (core + 1) * B2)
        m = dict(base)
        m["q_in"] = np.ascontiguousarray(queries[sl])
        m["k_in"] = np.ascontiguousarray(keys[sl])
        m["v_in"] = np.ascontiguousarray(values[sl])
        in_maps.append(m)
    res = bass_utils.run_bass_kernel_spmd(nc, in_maps, core_ids=list(range(NCORES)))
    out = np.empty((B, L, DM), np.float32)
    corr = np.empty((B, H, DK, L), np.float32)
    for core in range(NCORES):
        r = res.results[core]
        out[core * B2:(core + 1) * B2] = r["out"]
        corr[core * B2:(core + 1) * B2] = r["corr"].reshape(B2, H, DK, L)
    return out, corr
